# revision 1
# baseline (speedup 1.0000x reference)
"""Trainium2 Bass kernel v4 (v2 + fused epilogue) for nn_BlockPiecewiseLinear (histogram_binning).

Math (same reformulation as baseline, validated to ~4e-6):
    S    = softplus(slope)                      # [.., K+1]
    xs   = sort(x_pos, axis=-1)                 # [.., K]  (fp16)
    dS_r = S[r+1] - S[r]            (r = 0..K-1)
    c    = #{k: x_k <= q}           (EXACT fp32 compare on unsorted x)
    step'_r = 1[r < c]              (prefix mask from exact count)
    A    = sum_r step'_r * dS_r
    W    = sum_r step'_r * dS_r * xs_r
    ssel = (S[0]+EPS) + A
    out  = q*ssel - xs[0]*(S[0]+EPS) + xs[0] - W + y_bias

v2 layout: knot-major fp16 tiles [P, K, G] so every bitonic layer's
tensor_tensor runs in the DVE 2x packed mode (fp32 row-major is stuck at
1x).  Sort is the all-ascending bitonic variant (reversal merge + halving
layers) -> every layer is exactly 2 ops.  The exact count c is computed in
fp32 on the row-major x (contiguous), so fp16 rounding can never flip a
segment decision (which would cause O(dS) errors).  ScalarE does softplus
(exp + transposed ln) and the x transpose-cast; DVE does the rest.
"""

import numpy as np

import concourse.bass as bass
import concourse.bacc as bacc
import concourse.mybir as mybir
import concourse.tile as tile
from concourse.bass_utils import run_bass_kernel_spmd

F32 = mybir.dt.float32
F16 = mybir.dt.float16
Alu = mybir.AluOpType
Act = mybir.ActivationFunctionType

B, F, K = 4096, 512, 32
KP1 = K + 1
EPS = 1e-3
NCORES = 8
P = 128
G = 128  # rows per partition per tile


def _ap(t, off_elems, dims):
    """AP on tile-view t with extra element offset and free dims list."""
    v = t[:, :, :] if len(t.shape) == 3 else t[:, :]
    return bass.AP(tensor=v.tensor, offset=v.offset + off_elems, ap=[v.ap[0]] + dims)


def _sort_layers():
    """All-ascending bitonic: for k in 1,2,4,8,16: reversal layer (i <-> 2k-1-i
    in 2k-blocks) then halving layers j=k/2..1 (i <-> i+j in 2j-blocks)."""
    layers = []
    k = 1
    while k < K:
        layers.append(("rev", k))
        j = k // 2
        while j >= 1:
            layers.append(("half", j))
            j //= 2
        k *= 2
    return layers  # 15 layers


def build_nc(nloc, g=G):
    rows_per_tile = P * g
    ntiles = nloc // rows_per_tile
    assert ntiles * rows_per_tile == nloc

    nc = bacc.Bacc("TRN2", target_bir_lowering=False, debug=False)
    x_d = nc.declare_dram_parameter("x", [nloc, K], F32, isOutput=False)
    sl_d = nc.declare_dram_parameter("sl", [nloc, KP1], F32, isOutput=False)
    q_d = nc.declare_dram_parameter("q", [nloc], F32, isOutput=False)
    yb_d = nc.declare_dram_parameter("yb", [P, g], F32, isOutput=False)
    io_d = nc.declare_dram_parameter("io", [P, K * g], F16, isOutput=False)
    out_d = nc.declare_dram_parameter("out", [nloc], F32, isOutput=True)
    ss_d = nc.declare_dram_parameter("ssel", [nloc], F32, isOutput=True)

    xv = x_d[:, :].rearrange("(t p g) k -> t p g k", p=P, g=g)
    slv = sl_d[:, :].rearrange("(t p g) k -> t p g k", p=P, g=g)
    qv = q_d[:].rearrange("(t p g) -> t p g", p=P, g=g)
    outv = out_d[:].rearrange("(t p g) -> t p g", p=P, g=g)
    ssv = ss_d[:].rearrange("(t p g) -> t p g", p=P, g=g)

    layers = _sort_layers()

    with tile.TileContext(nc) as tc:
        with (
            tc.tile_pool(name="pcst", bufs=1) as pcst,
            tc.tile_pool(name="px", bufs=2) as px,
            tc.tile_pool(name="psl", bufs=2) as psl,
            tc.tile_pool(name="pq", bufs=2) as pq,
            tc.tile_pool(name="pS", bufs=2) as pS,
            tc.tile_pool(name="pxm", bufs=2) as pxm,
            tc.tile_pool(name="psort", bufs=3) as psort,
            tc.tile_pool(name="pst0", bufs=1) as pst0,
            tc.tile_pool(name="ptc", bufs=1) as ptc,
            tc.tile_pool(name="pstp", bufs=1) as pstp,
            tc.tile_pool(name="pdS", bufs=1) as pdS,
            tc.tile_pool(name="pmw", bufs=1) as pmw,
            tc.tile_pool(name="ptm", bufs=1) as ptm,
            tc.tile_pool(name="psm", bufs=1) as psm,
            tc.tile_pool(name="pout", bufs=3) as pout,
        ):
            yb_t = pcst.tile([P, g], F32, tag="yb")
            nc.sync.dma_start(out=yb_t[:, :], in_=yb_d[:, :])
            io_t = pcst.tile([P, K, g], F16, tag="io")
            nc.sync.dma_start(
                out=io_t[:, :, :], in_=io_d[:, :].rearrange("p (k g) -> p k g", g=g)
            )

            for t in range(ntiles):
                # ---------------- DMA loads ----------------
                x_t = px.tile([P, g, K], F32, tag="x")
                nc.sync.dma_start(out=x_t[:, :, :], in_=xv[t])
                sl_t = psl.tile([P, g, KP1], F32, tag="sl")
                nc.scalar.dma_start(out=sl_t[:, :, :], in_=slv[t])
                q_t = pq.tile([P, g], F32, tag="q")
                nc.sync.dma_start(out=q_t[:, :], in_=qv[t])

                # ---------------- ScalarE: softplus + x transpose-cast ----
                # S = ln(1 + exp(sl)); ln reads row-major, writes knot-major f16
                nc.scalar.activation(out=sl_t[:, :, :], in_=sl_t[:, :, :], func=Act.Exp)
                S_t = pS.tile([P, KP1, g], F16, tag="S")
                nc.scalar.activation(
                    out=_ap(S_t, 0, [[g, KP1], [1, g]]),
                    in_=_ap(sl_t, 0, [[1, KP1], [KP1, g]]),
                    func=Act.Ln,
                    bias=1.0,
                )
                # x f32 row-major -> f16 knot-major
                xm_t = pxm.tile([P, K, g], F16, tag="xm")
                nc.scalar.activation(
                    out=_ap(xm_t, 0, [[g, K], [1, g]]),
                    in_=_ap(x_t, 0, [[1, K], [K, g]]),
                    func=Act.Copy,
                )

                # ---------------- DVE: exact count c ----------------
                # step0 (row-major, f16) = 1[x <= q]; fp32 compare
                st0 = pst0.tile([P, g, K], F16, tag="st0")
                nc.vector.tensor_tensor(
                    out=st0[:, :, :],
                    in0=x_t[:, :, :],
                    in1=_ap(q_t, 0, [[1, g], [0, K]]),
                    op=Alu.is_le,
                )
                # 5-level pair tree over K (innermost) -> c [P, g] f16
                # levels write to disjoint offsets of tc1: L1@0(w16) L2@16(w8)
                # L3@24(w4) L4@28(w2); L5 -> contiguous c_sm [P, g]
                tc1 = ptc.tile([P, g, K], F16, tag="tc")
                nc.vector.tensor_tensor(
                    out=_ap(tc1, 0, [[K, g], [1, 16]]),
                    in0=_ap(st0, 0, [[K, g], [1, 16]]),
                    in1=_ap(st0, 16, [[K, g], [1, 16]]),
                    op=Alu.add,
                )
                nc.vector.tensor_tensor(
                    out=_ap(tc1, 16, [[K, g], [1, 8]]),
                    in0=_ap(tc1, 0, [[K, g], [1, 8]]),
                    in1=_ap(tc1, 8, [[K, g], [1, 8]]),
                    op=Alu.add,
                )
                nc.vector.tensor_tensor(
                    out=_ap(tc1, 24, [[K, g], [1, 4]]),
                    in0=_ap(tc1, 16, [[K, g], [1, 4]]),
                    in1=_ap(tc1, 20, [[K, g], [1, 4]]),
                    op=Alu.add,
                )
                nc.vector.tensor_tensor(
                    out=_ap(tc1, 28, [[K, g], [1, 2]]),
                    in0=_ap(tc1, 24, [[K, g], [1, 2]]),
                    in1=_ap(tc1, 26, [[K, g], [1, 2]]),
                    op=Alu.add,
                )
                c_sm = ptc.tile([P, g], F16, tag="csm")
                nc.vector.tensor_tensor(
                    out=c_sm[:, :],
                    in0=_ap(tc1, 28, [[K, g]]),
                    in1=_ap(tc1, 29, [[K, g]]),
                    op=Alu.add,
                )

                # step' (knot-major) = 1[iota < c]
                stp = pstp.tile([P, K, g], F16, tag="stp")
                nc.vector.tensor_tensor(
                    out=stp[:, :, :],
                    in0=io_t[:, :, :],
                    in1=bass.AP(
                        tensor=c_sm.tensor,
                        offset=c_sm[:, :].offset,
                        ap=[c_sm[:, :].ap[0], [0, K], [1, g]],
                    ),
                    op=Alu.is_lt,
                )

                # ---------------- DVE: bitonic sort (knot-major f16) ------
                cur = xm_t
                for kind, kk in layers:
                    dst = psort.tile([P, K, g], F16, tag="srt")
                    if kind == "rev":
                        bsz = 2 * kk
                        nb = K // bsz
                        in_lo = _ap(cur, 0, [[bsz * g, nb], [1, kk * g]])
                        if kk == 1:
                            in_hi = _ap(cur, g, [[bsz * g, nb], [1, g]])
                            o_max = _ap(dst, g, [[bsz * g, nb], [1, g]])
                        else:
                            in_hi = _ap(cur, (bsz - 1) * g, [[bsz * g, nb], [-g, kk], [1, g]])
                            o_max = _ap(dst, (bsz - 1) * g, [[bsz * g, nb], [-g, kk], [1, g]])
                        o_min = _ap(dst, 0, [[bsz * g, nb], [1, kk * g]])
                    else:
                        jj = kk
                        bsz = 2 * jj
                        nb = K // bsz
                        in_lo = _ap(cur, 0, [[bsz * g, nb], [1, jj * g]])
                        in_hi = _ap(cur, jj * g, [[bsz * g, nb], [1, jj * g]])
                        o_min = _ap(dst, 0, [[bsz * g, nb], [1, jj * g]])
                        o_max = _ap(dst, jj * g, [[bsz * g, nb], [1, jj * g]])
                    nc.vector.tensor_tensor(out=o_min, in0=in_lo, in1=in_hi, op=Alu.min)
                    nc.vector.tensor_tensor(out=o_max, in0=in_lo, in1=in_hi, op=Alu.max)
                    cur = dst
                xs_t = cur  # sorted ascending, [P, K, g] f16

                # ---------------- DVE: dS, m, w, tree-reduce ----------------
                dS_t = pdS.tile([P, K, g], F16, tag="dS")
                nc.vector.tensor_tensor(
                    out=dS_t[:, :, :],
                    in0=_ap(S_t, g, [[1, K * g]]),
                    in1=_ap(S_t, 0, [[1, K * g]]),
                    op=Alu.subtract,
                )
                mw = pmw.tile([P, 2, K, g], F16, tag="mw")
                m_v = _ap(mw, 0, [[1, K * g]])
                w_v = _ap(mw, K * g, [[1, K * g]])
                nc.vector.tensor_tensor(out=m_v, in0=stp[:, :, :], in1=dS_t[:, :, :], op=Alu.mult)
                nc.vector.tensor_tensor(out=w_v, in0=m_v, in1=xs_t[:, :, :], op=Alu.mult)
                # 5-level tree over K for both halves; last 2 levels in f32
                t16 = ptm.tile([P, 2, 16, g], F16, tag="t16")
                nc.vector.tensor_tensor(
                    out=_ap(t16, 0, [[16 * g, 2], [1, 16 * g]]),
                    in0=_ap(mw, 0, [[K * g, 2], [1, 16 * g]]),
                    in1=_ap(mw, 16 * g, [[K * g, 2], [1, 16 * g]]),
                    op=Alu.add,
                )
                t8 = ptm.tile([P, 2, 8, g], F16, tag="t8")
                nc.vector.tensor_tensor(
                    out=_ap(t8, 0, [[8 * g, 2], [1, 8 * g]]),
                    in0=_ap(t16, 0, [[16 * g, 2], [1, 8 * g]]),
                    in1=_ap(t16, 8 * g, [[16 * g, 2], [1, 8 * g]]),
                    op=Alu.add,
                )
                t4 = ptm.tile([P, 2, 4, g], F16, tag="t4")
                nc.vector.tensor_tensor(
                    out=_ap(t4, 0, [[4 * g, 2], [1, 4 * g]]),
                    in0=_ap(t8, 0, [[8 * g, 2], [1, 4 * g]]),
                    in1=_ap(t8, 4 * g, [[8 * g, 2], [1, 4 * g]]),
                    op=Alu.add,
                )
                t2 = psm.tile([P, 2, 2, g], F32, tag="t2")
                nc.vector.tensor_tensor(
                    out=_ap(t2, 0, [[2 * g, 2], [1, 2 * g]]),
                    in0=_ap(t4, 0, [[4 * g, 2], [1, 2 * g]]),
                    in1=_ap(t4, 2 * g, [[4 * g, 2], [1, 2 * g]]),
                    op=Alu.add,
                )
                t1 = psm.tile([P, 2, 1, g], F32, tag="t1")
                nc.vector.tensor_tensor(
                    out=_ap(t1, 0, [[g, 2], [1, g]]),
                    in0=_ap(t2, 0, [[2 * g, 2], [1, g]]),
                    in1=_ap(t2, g, [[2 * g, 2], [1, g]]),
                    op=Alu.add,
                )
                A_v = _ap(t1, 0, [[1, g]])
                W_v = _ap(t1, g, [[1, g]])

                # ---------------- epilogue (fused via STT) -----------------
                # ssel = (S0 + EPS) + A ; t1e = (S0 + (EPS-1)) * xmin
                # out  = q*ssel - t1e - W + yb
                sm = psm.tile([P, 2, g], F32, tag="sm")
                ss_t = pout.tile([P, g], F32, tag="ss")
                nc.vector.scalar_tensor_tensor(
                    out=ss_t[:, :], in0=_ap(S_t, 0, [[1, g]]), scalar=EPS,
                    in1=A_v, op0=Alu.add, op1=Alu.add,
                )
                t1e = _ap(sm, 0, [[1, g]])
                nc.vector.scalar_tensor_tensor(
                    out=t1e, in0=_ap(S_t, 0, [[1, g]]), scalar=EPS - 1.0,
                    in1=_ap(xs_t, 0, [[1, g]]), op0=Alu.add, op1=Alu.mult,
                )
                u = _ap(sm, g, [[1, g]])
                nc.vector.tensor_tensor(out=u, in0=q_t[:, :], in1=ss_t[:, :], op=Alu.mult)
                nc.vector.tensor_tensor(out=u, in0=u, in1=t1e, op=Alu.subtract)
                nc.vector.tensor_tensor(out=u, in0=u, in1=W_v, op=Alu.subtract)
                out_t = pout.tile([P, g], F32, tag="out")
                nc.vector.tensor_tensor(
                    out=out_t[:, :], in0=u, in1=yb_t[:, :], op=Alu.add
                )

                # ---------------- stores ----------------
                nc.sync.dma_start(out=outv[t], in_=out_t[:, :])
                nc.scalar.dma_start(out=ssv[t], in_=ss_t[:, :])
    nc.compile()
    return nc


_NC_CACHE = {}


def _get_nc(nloc, g=G):
    key = (nloc, g)
    if key not in _NC_CACHE:
        _NC_CACHE[key] = build_nc(nloc, g)
    return _NC_CACHE[key]


def make_iota():
    io = np.broadcast_to(
        np.arange(K, dtype=np.float16)[None, :, None], (P, K, G)
    )
    return np.ascontiguousarray(io.reshape(P, K * G))


def make_in_maps(inputs, x_pos, slope, y_bias):
    b, f = inputs.shape
    bloc = b // NCORES
    nloc = bloc * f
    yb_exp = np.ascontiguousarray(
        np.tile(y_bias.astype(np.float32)[:, 0], (P * G) // f).reshape(P, G)
    )
    io = make_iota()
    in_maps = []
    for c in range(NCORES):
        sl_b = slice(c * bloc, (c + 1) * bloc)
        in_maps.append(
            {
                "x": np.ascontiguousarray(x_pos[sl_b].astype(np.float32).reshape(nloc, K)),
                "sl": np.ascontiguousarray(slope[sl_b].astype(np.float32).reshape(nloc, KP1)),
                "q": np.ascontiguousarray(inputs[sl_b].astype(np.float32).reshape(nloc)),
                "yb": yb_exp,
                "io": io,
            }
        )
    return in_maps, bloc, nloc


def kernel(inputs, x_pos, slope, y_bias):
    inputs = np.ascontiguousarray(np.asarray(inputs, dtype=np.float32))
    x_pos = np.ascontiguousarray(np.asarray(x_pos, dtype=np.float32))
    slope = np.ascontiguousarray(np.asarray(slope, dtype=np.float32))
    y_bias = np.ascontiguousarray(np.asarray(y_bias, dtype=np.float32))

    in_maps, bloc, nloc = make_in_maps(inputs, x_pos, slope, y_bias)
    b, f = inputs.shape
    nc = _get_nc(nloc)
    res = run_bass_kernel_spmd(nc, in_maps, list(range(NCORES)))
    outs = np.concatenate(
        [res.results[c]["out"].reshape(bloc, f) for c in range(NCORES)], axis=0
    )
    ssel = np.concatenate(
        [res.results[c]["ssel"].reshape(bloc, f) for c in range(NCORES)], axis=0
    )
    return outs, ssel



# revision 2
# speedup vs baseline: 1.0084x; 1.0084x over previous
"""Trainium2 Bass kernel v6 for nn_BlockPiecewiseLinear (histogram_binning).

v5b + pair-of-tiles double-width processing: two G=128 tiles share each DVE
instruction (sort, trees, masks run on [P, K, 2G] tiles), halving instruction
count; Exp/Ln batched per pair to cut ACT_TABLE_LOAD thrash; scratch tiles
folded (in-place count tree on st0, stp/dS folded into the mw buffer).

Math (validated):
    S    = softplus(slope)
    xs   = sort(x_pos, axis=-1)            (fp16, odd-even-merge network)
    c    = #{k: x_k <= q}                  (EXACT fp32 compare)
    m_r  = 1[r < c] * (S[r+1] - S[r]) ; A = sum m ; W = sum m*xs
    ssel = (S[0]+EPS) + A
    out  = q*ssel - xs[0]*(S[0]+EPS) + xs[0] - W + y_bias
"""

import numpy as np

import concourse.bass as bass
import concourse.bacc as bacc
import concourse.mybir as mybir
import concourse.tile as tile
from concourse.bass_utils import run_bass_kernel_spmd

F32 = mybir.dt.float32
F16 = mybir.dt.float16
Alu = mybir.AluOpType
Act = mybir.ActivationFunctionType

B, F, K = 4096, 512, 32
KP1 = K + 1
EPS = 1e-3
NCORES = 8
P = 128
G = 128
G2 = 2 * G  # double-width free dim

OEM_LAYERS = [
    (1, [(2, 16)], 0, 16, None),
    (2, [(4, 8), (1, 2)], 0, 16, None),
    (1, [(4, 8)], 1, 8, "ip"),
    (4, [(8, 4), (1, 4)], 0, 16, None),
    (2, [(8, 4), (1, 2)], 2, 8, "ip"),
    (1, [(8, 4), (2, 3)], 1, 12, (0, [(8, 4), (7, 2)])),
    (8, [(16, 2), (1, 8)], 0, 16, None),
    (4, [(16, 2), (1, 4)], 4, 8, "ip"),
    (2, [(16, 2), (4, 3), (1, 2)], 2, 12, (0, [(16, 2), (14, 2), (1, 2)])),
    (1, [(16, 2), (2, 7)], 1, 14, (0, [(16, 2), (15, 2)])),
    (16, [(1, 16)], 0, 16, None),
    (8, [(1, 8)], 8, 8, "ip"),
    (4, [(8, 3), (1, 4)], 4, 12, (0, [(28, 2), (1, 4)])),
    (2, [(4, 7), (1, 2)], 2, 14, (0, [(30, 2), (1, 2)])),
    (1, [(2, 15)], 1, 15, (0, [(31, 2)])),
]


def _ap(t, off_elems, dims):
    v = t[tuple([slice(None)] * len(t.shape))]
    return bass.AP(tensor=v.tensor, offset=v.offset + off_elems, ap=[v.ap[0]] + dims)


def _knot_ap(t, start_knots, knot_dims, w):
    """AP over [P, K, w] tile: knot offsets in units of w, with [1, w] tail."""
    dims = [[s * w, c] for (s, c) in knot_dims] + [[1, w]]
    return _ap(t, start_knots * w, dims)


def build_nc(nloc):
    rows_per_tile = P * G
    ntiles = nloc // rows_per_tile
    assert ntiles % 2 == 0
    npairs = ntiles // 2

    nc = bacc.Bacc("TRN2", target_bir_lowering=False, debug=False)
    x_d = nc.declare_dram_parameter("x", [nloc, K], F32, isOutput=False)
    sl_d = nc.declare_dram_parameter("sl", [nloc, KP1], F32, isOutput=False)
    q_d = nc.declare_dram_parameter("q", [nloc], F32, isOutput=False)
    yb_d = nc.declare_dram_parameter("yb", [P, G2], F16, isOutput=False)
    io_d = nc.declare_dram_parameter("io", [P, K * 2], F16, isOutput=False)
    out_d = nc.declare_dram_parameter("out", [nloc], F32, isOutput=True)
    ss_d = nc.declare_dram_parameter("ssel", [nloc], F32, isOutput=True)

    xv = x_d[:, :].rearrange("(t p g) k -> t p g k", p=P, g=G)
    slv = sl_d[:, :].rearrange("(t p g) k -> t p g k", p=P, g=G)
    qv = q_d[:].rearrange("(t p g) -> t p g", p=P, g=G)
    outv = out_d[:].rearrange("(t p g) -> t p g", p=P, g=G)
    ssv = ss_d[:].rearrange("(t p g) -> t p g", p=P, g=G)

    with tile.TileContext(nc) as tc:
        with (
            tc.tile_pool(name="pcst", bufs=1) as pcst,
            tc.tile_pool(name="px", bufs=1) as px,
            tc.tile_pool(name="psl", bufs=1) as psl,
            tc.tile_pool(name="pq", bufs=2) as pq,
            tc.tile_pool(name="pS", bufs=2) as pS,
            tc.tile_pool(name="pxm", bufs=2) as pxm,
            tc.tile_pool(name="psA", bufs=1) as psA,
            tc.tile_pool(name="psB", bufs=1) as psB,
            tc.tile_pool(name="pscr", bufs=1) as pscr,
            tc.tile_pool(name="pcsm", bufs=1) as pcsm,
            tc.tile_pool(name="pmw", bufs=1) as pmw,
            tc.tile_pool(name="psm", bufs=1) as psm,
            tc.tile_pool(name="pout", bufs=1) as pout,
        ):
            yb_t = pcst.tile([P, G2], F16, tag="yb")
            nc.sync.dma_start(out=yb_t[:, :], in_=yb_d[:, :])
            io_t = pcst.tile([P, K, 2], F16, tag="io")
            nc.sync.dma_start(
                out=io_t[:, :, :], in_=io_d[:, :].rearrange("p (k g) -> p k g", g=2)
            )

            for pr in range(npairs):
                t0, t1 = 2 * pr, 2 * pr + 1
                # ---------------- DMA loads ----------------
                x0 = px.tile([P, G, K], F32, tag="x0")
                nc.sync.dma_start(out=x0[:, :, :], in_=xv[t0])
                x1 = px.tile([P, G, K], F32, tag="x1")
                nc.sync.dma_start(out=x1[:, :, :], in_=xv[t1])
                sl0 = psl.tile([P, G, KP1], F32, tag="sl0")
                nc.scalar.dma_start(out=sl0[:, :, :], in_=slv[t0])
                sl1 = psl.tile([P, G, KP1], F32, tag="sl1")
                nc.scalar.dma_start(out=sl1[:, :, :], in_=slv[t1])
                q2 = pq.tile([P, G2], F32, tag="q")
                nc.sync.dma_start(out=_ap(q2, 0, [[1, G]]), in_=qv[t0])
                nc.sync.dma_start(out=_ap(q2, G, [[1, G]]), in_=qv[t1])

                # ---------------- ScalarE ----------------
                # x transpose-casts first (feed the sort), then Exp/Exp Ln/Ln
                xm2 = pxm.tile([P, K, G2], F16, tag="xm")
                nc.scalar.activation(
                    out=_ap(xm2, 0, [[G2, K], [1, G]]),
                    in_=_ap(x0, 0, [[1, K], [K, G]]),
                    func=Act.Copy,
                )
                nc.scalar.activation(
                    out=_ap(xm2, G, [[G2, K], [1, G]]),
                    in_=_ap(x1, 0, [[1, K], [K, G]]),
                    func=Act.Copy,
                )
                nc.scalar.activation(out=sl0[:, :, :], in_=sl0[:, :, :], func=Act.Exp)
                nc.scalar.activation(out=sl1[:, :, :], in_=sl1[:, :, :], func=Act.Exp)
                S2 = pS.tile([P, KP1, G2], F16, tag="S")
                nc.scalar.activation(
                    out=_ap(S2, 0, [[G2, KP1], [1, G]]),
                    in_=_ap(sl0, 0, [[1, KP1], [KP1, G]]),
                    func=Act.Ln,
                    bias=1.0,
                )
                nc.scalar.activation(
                    out=_ap(S2, G, [[G2, KP1], [1, G]]),
                    in_=_ap(sl1, 0, [[1, KP1], [KP1, G]]),
                    func=Act.Ln,
                    bias=1.0,
                )

                # ---------------- DVE: exact count (in-place tree) --------
                st2 = psB.tile([P, K, G2], F16, tag="sB")
                nc.vector.tensor_tensor(
                    out=_ap(st2, 0, [[K, G], [1, K]]),
                    in0=x0[:, :, :],
                    in1=_ap(q2, 0, [[1, G], [0, K]]),
                    op=Alu.is_le,
                )
                nc.vector.tensor_tensor(
                    out=_ap(st2, G * K, [[K, G], [1, K]]),
                    in0=x1[:, :, :],
                    in1=_ap(q2, G, [[1, G], [0, K]]),
                    op=Alu.is_le,
                )
                # in-place tree on st2 over K (both tiles at once: G2 rows)
                nc.vector.tensor_tensor(
                    out=_ap(st2, 0, [[K, G2], [1, 16]]),
                    in0=_ap(st2, 0, [[K, G2], [1, 16]]),
                    in1=_ap(st2, 16, [[K, G2], [1, 16]]),
                    op=Alu.add,
                )
                nc.vector.tensor_tensor(
                    out=_ap(st2, 0, [[K, G2], [1, 8]]),
                    in0=_ap(st2, 0, [[K, G2], [1, 8]]),
                    in1=_ap(st2, 8, [[K, G2], [1, 8]]),
                    op=Alu.add,
                )
                nc.vector.tensor_tensor(
                    out=_ap(st2, 0, [[K, G2], [1, 4]]),
                    in0=_ap(st2, 0, [[K, G2], [1, 4]]),
                    in1=_ap(st2, 4, [[K, G2], [1, 4]]),
                    op=Alu.add,
                )
                nc.vector.tensor_tensor(
                    out=_ap(st2, 0, [[K, G2], [1, 2]]),
                    in0=_ap(st2, 0, [[K, G2], [1, 2]]),
                    in1=_ap(st2, 2, [[K, G2], [1, 2]]),
                    op=Alu.add,
                )
                c_sm = pcsm.tile([P, G2], F16, tag="csm")
                nc.vector.tensor_tensor(
                    out=c_sm[:, :],
                    in0=_ap(st2, 0, [[K, G2]]),
                    in1=_ap(st2, 1, [[K, G2]]),
                    op=Alu.add,
                )

                # ---------------- DVE: OEM sort on [P, K, G2] -------------
                bufA = psA.tile([P, K, G2], F16, tag="sA")
                bufB = psB.tile([P, K, G2], F16, tag="sB")  # aliases st2 memory
                scr = pscr.tile([P, 8, G2], F16, tag="scr")
                cur = xm2
                nxt = bufA
                other = bufB
                for gap, idims, start, npr, unc in OEM_LAYERS:
                    in_lo = _knot_ap(cur, start, idims, G2)
                    in_hi = _knot_ap(cur, start + gap, idims, G2)
                    if unc == "ip":
                        s_ap = _ap(scr, 0, [[G2, npr], [1, G2]])
                        nc.vector.tensor_tensor(out=s_ap, in0=in_lo, in1=in_hi, op=Alu.max)
                        nc.vector.tensor_tensor(out=in_lo, in0=in_lo, in1=in_hi, op=Alu.min)
                        nc.vector.tensor_copy(out=in_hi, in_=s_ap)
                    else:
                        dst = nxt
                        o_min = _knot_ap(dst, start, idims, G2)
                        o_max = _knot_ap(dst, start + gap, idims, G2)
                        nc.vector.tensor_tensor(out=o_min, in0=in_lo, in1=in_hi, op=Alu.min)
                        nc.vector.tensor_tensor(out=o_max, in0=in_lo, in1=in_hi, op=Alu.max)
                        if unc is not None:
                            ustart, udims = unc
                            nc.vector.tensor_copy(
                                out=_knot_ap(dst, ustart, udims, G2),
                                in_=_knot_ap(cur, ustart, udims, G2),
                            )
                        if cur is xm2:
                            cur, nxt, other = dst, other, bufA
                        else:
                            cur, nxt = dst, cur
                xs2 = cur  # sorted ascending, [P, K, G2] f16

                # ---------------- DVE: dS, stp, m, w, in-place tree -------
                mw = pmw.tile([P, 2, K, G2], F16, tag="mw")
                mv0 = _ap(mw, 0, [[1, K * G2]])        # half 0: m (dS first)
                wv0 = _ap(mw, K * G2, [[1, K * G2]])   # half 1: stp then w
                # dS -> mw[0]
                nc.vector.tensor_tensor(
                    out=mv0, in0=_ap(S2, G2, [[1, K * G2]]),
                    in1=_ap(S2, 0, [[1, K * G2]]), op=Alu.subtract,
                )
                # stp -> mw[1]; iota [P,K,G] broadcast across both halves
                nc.vector.tensor_tensor(
                    out=wv0,
                    in0=_ap(io_t, 0, [[2, K], [0, G], [1, 2]]),
                    in1=bass.AP(
                        tensor=c_sm.tensor, offset=c_sm[:, :].offset,
                        ap=[c_sm[:, :].ap[0], [0, K], [1, G2]],
                    ),
                    op=Alu.is_lt,
                )
                # m = stp * dS (in-place into mw[0])
                nc.vector.tensor_tensor(out=mv0, in0=wv0, in1=mv0, op=Alu.mult)
                # w = m * xs (into mw[1], overwrites stp)
                nc.vector.tensor_tensor(out=wv0, in0=mv0, in1=_ap(xs2, 0, [[1, K * G2]]), op=Alu.mult)
                # in-place 2-in-1 tree over K (f16 until last level -> f32)
                nc.vector.tensor_tensor(
                    out=_ap(mw, 0, [[K * G2, 2], [1, 16 * G2]]),
                    in0=_ap(mw, 0, [[K * G2, 2], [1, 16 * G2]]),
                    in1=_ap(mw, 16 * G2, [[K * G2, 2], [1, 16 * G2]]),
                    op=Alu.add,
                )
                nc.vector.tensor_tensor(
                    out=_ap(mw, 0, [[K * G2, 2], [1, 8 * G2]]),
                    in0=_ap(mw, 0, [[K * G2, 2], [1, 8 * G2]]),
                    in1=_ap(mw, 8 * G2, [[K * G2, 2], [1, 8 * G2]]),
                    op=Alu.add,
                )
                nc.vector.tensor_tensor(
                    out=_ap(mw, 0, [[K * G2, 2], [1, 4 * G2]]),
                    in0=_ap(mw, 0, [[K * G2, 2], [1, 4 * G2]]),
                    in1=_ap(mw, 4 * G2, [[K * G2, 2], [1, 4 * G2]]),
                    op=Alu.add,
                )
                nc.vector.tensor_tensor(
                    out=_ap(mw, 0, [[K * G2, 2], [1, 2 * G2]]),
                    in0=_ap(mw, 0, [[K * G2, 2], [1, 2 * G2]]),
                    in1=_ap(mw, 2 * G2, [[K * G2, 2], [1, 2 * G2]]),
                    op=Alu.add,
                )
                t1w = psm.tile([P, 2, G2], F32, tag="t1")
                nc.vector.tensor_tensor(
                    out=_ap(t1w, 0, [[G2, 2], [1, G2]]),
                    in0=_ap(mw, 0, [[K * G2, 2], [1, G2]]),
                    in1=_ap(mw, G2, [[K * G2, 2], [1, G2]]),
                    op=Alu.add,
                )
                A_v = _ap(t1w, 0, [[1, G2]])
                W_v = _ap(t1w, G2, [[1, G2]])

                # ---------------- epilogue (double width) -----------------
                sm = psm.tile([P, 2, G2], F32, tag="sm")
                ss_t = pout.tile([P, G2], F32, tag="ss")
                S0_v = _ap(S2, 0, [[1, G2]])
                nc.vector.scalar_tensor_tensor(
                    out=ss_t[:, :], in0=S0_v, scalar=EPS,
                    in1=A_v, op0=Alu.add, op1=Alu.add,
                )
                t1e = _ap(sm, 0, [[1, G2]])
                nc.vector.scalar_tensor_tensor(
                    out=t1e, in0=S0_v, scalar=EPS - 1.0,
                    in1=_ap(xs2, 0, [[1, G2]]), op0=Alu.add, op1=Alu.mult,
                )
                z_v = _ap(sm, G2, [[1, G2]])
                nc.vector.tensor_tensor(out=z_v, in0=t1e, in1=W_v, op=Alu.add)
                nc.vector.tensor_tensor(out=z_v, in0=z_v, in1=yb_t[:, :], op=Alu.subtract)
                out_t = pout.tile([P, G2], F32, tag="out")
                nc.vector.tensor_tensor(out=out_t[:, :], in0=q2[:, :], in1=ss_t[:, :], op=Alu.mult)
                nc.vector.tensor_tensor(out=out_t[:, :], in0=out_t[:, :], in1=z_v, op=Alu.subtract)

                # ---------------- stores ----------------
                nc.sync.dma_start(out=outv[t0], in_=_ap(out_t, 0, [[1, G]]))
                nc.sync.dma_start(out=outv[t1], in_=_ap(out_t, G, [[1, G]]))
                nc.scalar.dma_start(out=ssv[t0], in_=_ap(ss_t, 0, [[1, G]]))
                nc.scalar.dma_start(out=ssv[t1], in_=_ap(ss_t, G, [[1, G]]))
    nc.compile()
    return nc


_NC_CACHE = {}


def _get_nc(nloc):
    if nloc not in _NC_CACHE:
        _NC_CACHE[nloc] = build_nc(nloc)
    return _NC_CACHE[nloc]


def make_iota():
    io = np.broadcast_to(
        np.arange(K, dtype=np.float16)[None, :, None], (P, K, 2)
    )
    return np.ascontiguousarray(io.reshape(P, K * 2))


def make_in_maps(inputs, x_pos, slope, y_bias):
    b, f = inputs.shape
    bloc = b // NCORES
    nloc = bloc * f
    yb_exp = np.ascontiguousarray(
        np.tile(y_bias.astype(np.float16)[:, 0], (P * G2) // f).reshape(P, G2)
    )
    io = make_iota()
    in_maps = []
    for c in range(NCORES):
        sl_b = slice(c * bloc, (c + 1) * bloc)
        in_maps.append(
            {
                "x": np.ascontiguousarray(x_pos[sl_b].astype(np.float32).reshape(nloc, K)),
                "sl": np.ascontiguousarray(slope[sl_b].astype(np.float32).reshape(nloc, KP1)),
                "q": np.ascontiguousarray(inputs[sl_b].astype(np.float32).reshape(nloc)),
                "yb": yb_exp,
                "io": io,
            }
        )
    return in_maps, bloc, nloc


def kernel(inputs, x_pos, slope, y_bias):
    inputs = np.ascontiguousarray(np.asarray(inputs, dtype=np.float32))
    x_pos = np.ascontiguousarray(np.asarray(x_pos, dtype=np.float32))
    slope = np.ascontiguousarray(np.asarray(slope, dtype=np.float32))
    y_bias = np.ascontiguousarray(np.asarray(y_bias, dtype=np.float32))

    in_maps, bloc, nloc = make_in_maps(inputs, x_pos, slope, y_bias)
    b, f = inputs.shape
    nc = _get_nc(nloc)
    res = run_bass_kernel_spmd(nc, in_maps, list(range(NCORES)))
    outs = np.concatenate(
        [res.results[c]["out"].reshape(bloc, f) for c in range(NCORES)], axis=0
    )
    ssel = np.concatenate(
        [res.results[c]["ssel"].reshape(bloc, f) for c in range(NCORES)], axis=0
    )
    return outs, ssel
